# revision 14
# baseline (speedup 1.0000x reference)
"""Trainium2 Bass kernel for the nn_MultiHeadAttention problem.

Data-parallel over batch: each of the 8 NeuronCores processes one batch
element independently (no collectives).

Mask compaction: the host gathers only the valid query/key positions
(QMask/KMask true), padded to a multiple of 128; queries are capped at
512 (the rare overflow queries are computed exactly on the host).

The device program is engine-balanced so the PE array streams near
continuously (keeping the HAM clock gate at K=8/8, 2.4 GHz):

  PE     proj (fused [Q|K] rhs) + v-proj, scores (64-contraction),
         PV (65-row: 64 d + denominator), final out-projection
         (128-contraction, accumulated A+B in PSUM).
  Scalar EXP only (batched over PAIRS of score k-tiles, 2 PSUM banks
         per op) + one denominator copy per chunk.
  DVE    all projection PSUM evacuations, ct evacuation, second
         denominator copy, reciprocals, last-batch normalize muls.
  GpSimd normalize multiplies for batches 0/1 (off critical path),
         small const DMAs.
  Sync   all data DMAs (inputs, denominator gathers, reciprocal
         DRAM-bounce broadcasts, Y output).

Emission is software-pipelined: chunk c+1's projections are emitted
between chunk c's score blocks so the PE never waits on the EXP
pipeline; the final projection is interleaved with chunk 7's
scores/PV so the last-batch normalize chain is hidden.
"""

import math
import os
import sys

import numpy as np

try:
    import concourse  # noqa: F401
except ImportError:  # pragma: no cover
    for _p in ("/opt/trn_rl_repo", os.path.expanduser("~/.axon_site/_ro/trn_rl_repo")):
        if os.path.isdir(_p) and _p not in sys.path:
            sys.path.insert(0, _p)

import ml_dtypes

import concourse.bass as bass
import concourse.tile as tile
from concourse import bacc, mybir

B, L, E, H, D = 8, 1024, 1024, 16, 64
P = 128          # partitions
NCH = E // P     # 8 e-chunks (2 heads each)
F32 = mybir.dt.float32
BF16 = mybir.dt.bfloat16

# normalize batches: (head range start, end, after-chunk)
NORM_BATCHES = [(0, 8, 3), (8, 14, 6), (14, 16, 7)]


def _chunks(n, step=512):
    return [(s, min(s + step, n)) for s in range(0, n, step)]


def build_bass(ntq, ntk):
    Lq, Lk = ntq * P, ntk * P
    NG = 2 * ntk                # score tile units per chunk (2 heads)
    nc = bacc.Bacc(None, target_bir_lowering=False, debug=False)

    QT = nc.declare_dram_parameter("QT", [E, Lq], BF16, isOutput=False)
    KT = nc.declare_dram_parameter("KT", [E, Lk], BF16, isOutput=False)
    VT = nc.declare_dram_parameter("VT", [E, Lk], BF16, isOutput=False)
    W2 = nc.declare_dram_parameter("W2", [P, NCH, P], BF16, isOutput=False)
    OB = nc.declare_dram_parameter("OB", [E, E], BF16, isOutput=False)
    KM = nc.declare_dram_parameter("KM", [P, ntk], F32, isOutput=False)
    Y = nc.declare_dram_parameter("Y", [Lq, E], BF16, isOutput=True)
    rbounce = nc.dram_tensor("rbounce", [H, Lq], BF16)

    with tile.TileContext(nc) as tc:
        with (
            tc.tile_pool(name="singles", bufs=1) as singles,
            tc.tile_pool(name="qkT", bufs=2) as qkT,
            tc.tile_pool(name="vaug", bufs=2) as vaug,
            tc.tile_pool(name="ppool", bufs=2) as ppool,
            tc.tile_pool(name="ystage", bufs=2) as ystage,
            tc.tile_pool(name="bcpool", bufs=8) as bcpool,
            tc.tile_pool(name="dtpool", bufs=4) as dtpool,
            tc.tile_pool(name="psbig", bufs=2, space="PSUM") as psbig,
            tc.tile_pool(name="pspv", bufs=2, space="PSUM") as pspv,
            tc.tile_pool(name="pssmall", bufs=2, space="PSUM") as pssmall,
        ):
            # --- persistent SBUF tensors -------------------------------
            qts = singles.tile([P, NCH, Lq], BF16)
            kts = singles.tile([P, NCH, Lk], BF16)
            vts = singles.tile([P, NCH, Lk], BF16)
            obs = singles.tile([P, NCH, E], BF16)
            w2s = singles.tile([P, NCH, P], BF16)
            kms = singles.tile([P, ntk], F32)
            ct = singles.tile([P, NCH, Lq], BF16)
            dstacks = []
            rstacks = []
            for bi, (h0, h1, _) in enumerate(NORM_BATCHES):
                ds = singles.tile([(h1 - h0) * ntq, P], F32, tag=f"ds{bi}")
                rs = singles.tile([(h1 - h0) * ntq, P], BF16, tag=f"rs{bi}")
                dstacks.append(ds)
                rstacks.append(rs)

            # --- input DMAs (chunk 0 first so compute can start) -------
            nc.gpsimd.dma_start(out=w2s[:], in_=W2[:])
            nc.gpsimd.dma_start(out=kms[:], in_=KM[:])
            for c in (0,):
                nc.sync.dma_start(out=qts[:, c, :], in_=QT[c * P:(c + 1) * P, :])
                nc.sync.dma_start(out=kts[:, c, :], in_=KT[c * P:(c + 1) * P, :])
                nc.sync.dma_start(out=vts[:, c, :], in_=VT[c * P:(c + 1) * P, :])
            # PE warmup: ~3.4us of dummy matmuls opens the HAM clock gate
            # while the remaining input DMAs land
            warm = singles.tile([P, 512], BF16)
            nc.vector.memset(warm[:], 0.0)
            for wi in range(3):
                wps = pssmall.tile([P, 512], F32, tag="small")
                nc.tensor.matmul(out=wps[:], lhsT=warm[:, 0:128], rhs=warm[:],
                                 start=True, stop=True)
            for c in range(1, NCH):
                nc.sync.dma_start(out=qts[:, c, :], in_=QT[c * P:(c + 1) * P, :])
                nc.sync.dma_start(out=kts[:, c, :], in_=KT[c * P:(c + 1) * P, :])
                nc.sync.dma_start(out=vts[:, c, :], in_=VT[c * P:(c + 1) * P, :])
            for c in range(NCH):
                nc.sync.dma_start(out=obs[:, c, :], in_=OB[c * P:(c + 1) * P, :])

            # --- helpers ----------------------------------------------
            def emit_proj_qk_main(c):
                """First two 512-col pieces of the fused [Q|K] projection."""
                qkt2 = qkT.tile([P, Lq + Lk], BF16, tag="qkt2")
                pieces = _chunks(Lq + Lk)
                for s0, s1 in pieces[:2]:
                    ps = pssmall.tile([P, 512], F32, tag="small")
                    if s1 <= Lq:
                        nc.tensor.matmul(out=ps[:, 0:s1 - s0], lhsT=w2s[:, c, :],
                                         rhs=qts[:, c, s0:s1], start=True, stop=True)
                    elif s0 >= Lq:
                        nc.tensor.matmul(out=ps[:, 0:s1 - s0], lhsT=w2s[:, c, :],
                                         rhs=kts[:, c, s0 - Lq:s1 - Lq],
                                         start=True, stop=True)
                    else:
                        mid = Lq - s0
                        nc.tensor.matmul(out=ps[:, 0:mid], lhsT=w2s[:, c, :],
                                         rhs=qts[:, c, s0:Lq], start=True, stop=True)
                        nc.tensor.matmul(out=ps[:, mid:s1 - s0], lhsT=w2s[:, c, :],
                                         rhs=kts[:, c, 0:s1 - Lq], start=True, stop=True)
                    nc.vector.tensor_copy(qkt2[:, s0:s1], ps[:, 0:s1 - s0])
                return qkt2, pieces

            def emit_proj_rest(c, qkt2, pieces):
                """Remaining qk piece + v-projection, packed into two psum
                tiles, evacuated on DVE."""
                v2 = vaug.tile([P, ntk, 2, 66], BF16, tag="v2")
                # tile A: qk remainder + tail v-tiles; tile B: first 4 v-tiles
                rem = pieces[2:]
                ktail = list(range(4, ntk))
                psA = None
                if rem or ktail:
                    psA = pssmall.tile([P, 512], F32, tag="small")
                off = 0
                if rem:
                    s0, s1 = rem[0]
                    if s0 >= Lq:
                        nc.tensor.matmul(out=psA[:, 0:s1 - s0], lhsT=w2s[:, c, :],
                                         rhs=kts[:, c, s0 - Lq:s1 - Lq],
                                         start=True, stop=True)
                    else:
                        mid = Lq - s0
                        nc.tensor.matmul(out=psA[:, 0:mid], lhsT=w2s[:, c, :],
                                         rhs=qts[:, c, s0:Lq], start=True, stop=True)
                        nc.tensor.matmul(out=psA[:, mid:s1 - s0], lhsT=w2s[:, c, :],
                                         rhs=kts[:, c, 0:s1 - Lq], start=True, stop=True)
                    off = s1 - s0
                for i, t in enumerate(ktail):
                    nc.tensor.matmul(
                        out=psA[:, off + i * P: off + (i + 1) * P],
                        lhsT=vts[:, c, t * P:(t + 1) * P],
                        rhs=w2s[:, c, :], start=True, stop=True)
                nk_b = min(4, ntk)
                psB = pssmall.tile([P, 512], F32, tag="small")
                for t in range(nk_b):
                    nc.tensor.matmul(
                        out=psB[:, t * P:(t + 1) * P],
                        lhsT=vts[:, c, t * P:(t + 1) * P],
                        rhs=w2s[:, c, :], start=True, stop=True)
                # evacuations (DVE)
                if rem:
                    s0, s1 = rem[0]
                    nc.vector.tensor_copy(qkt2[:, s0:s1], psA[:, 0:off])
                if ktail:
                    src = psA[:, off:off + len(ktail) * P].rearrange(
                        "p (t two d) -> p t two d", two=2, d=64)
                    nc.vector.tensor_copy(v2[:, 4:ntk, :, 0:64], src)
                srcB = psB[:, 0:nk_b * P].rearrange(
                    "p (t two d) -> p t two d", two=2, d=64)
                nc.vector.tensor_copy(v2[:, 0:nk_b, :, 0:64], srcB)
                # denominator "ones" columns = slot-validity mask
                nc.gpsimd.tensor_copy(v2[:, :, 0, 64], kms[:, :])
                nc.gpsimd.tensor_copy(v2[:, :, 1, 64], kms[:, :])
                return v2

            def emit_scores(c, qkt2, pt, hf, sps_live):
                """Score matmuls for head-half hf; EXP emitted per pair."""
                hq = qkt2[64 * hf:64 * hf + 64, 0:Lq]
                for t in range(ntk):
                    g = hf * ntk + t
                    if g % 2 == 0:
                        sps_live[0] = psbig.tile([P, 2, Lq], F32, tag="big",
                                                 name="sp",
                                                 padded_shape=[P, 2, 512])
                    sp = sps_live[0]
                    hk = qkt2[64 * hf:64 * hf + 64,
                              Lq + t * P:Lq + (t + 1) * P]
                    nc.tensor.matmul(out=sp[:, g % 2, :], lhsT=hk, rhs=hq,
                                     start=True, stop=True)
                    if g % 2 == 1:
                        nc.scalar.activation(
                            out=pt[:, g - 1:g + 1, :], in_=sp[:, :, 0:Lq],
                            func=mybir.ActivationFunctionType.Exp, scale=0.125)
                    elif g == NG - 1:
                        nc.scalar.activation(
                            out=pt[:, g:g + 1, :], in_=sp[:, 0:1, 0:Lq],
                            func=mybir.ActivationFunctionType.Exp, scale=0.125)

            def emit_pv(c, pt, v2, hf):
                """PV accumulation + ct/denominator evacuation."""
                h = 2 * c + hf
                pv = pspv.tile([65, Lq], F32, tag="pv", padded_shape=[P, 512])
                for kt in range(ntk):
                    nc.tensor.matmul(
                        out=pv[:, 0:Lq],
                        lhsT=v2[:, kt, hf, 0:65],
                        rhs=pt[:, hf * ntk + kt, :],
                        start=(kt == 0), stop=(kt == ntk - 1))
                bi = next(i for i, (a, b, _) in enumerate(NORM_BATCHES)
                          if a <= h < b)
                hrel = h - NORM_BATCHES[bi][0]
                dtmp = dtpool.tile([65, Lq], F32, tag="dt")
                nc.vector.tensor_copy(dtmp[64:65, :], pv[64:65, 0:Lq])
                nc.sync.dma_start(
                    out=dstacks[bi][hrel * ntq:(hrel + 1) * ntq, :],
                    in_=dtmp[64:65, :])
                nc.vector.tensor_copy(ct[64 * hf:64 * hf + 64, c, :],
                                      pv[0:64, 0:Lq])

            def normalize_phase1(bi):
                """recip + DRAM-bounce broadcast DMAs (no engine muls)."""
                h0, h1, _ = NORM_BATCHES[bi]
                with nc.allow_low_precision(reason="softmax recip bf16"):
                    nc.vector.reciprocal(out=rstacks[bi][:], in_=dstacks[bi][:])
                nc.sync.dma_start(out=rbounce[h0:h1, :], in_=rstacks[bi][:])
                pairs = []
                for h in range(h0, h1):
                    bcs = bcpool.tile([P, Lq], BF16, tag="bcs")
                    src = rbounce[h:h + 1, :]
                    bc_in = bass.AP(
                        tensor=src.tensor, offset=src.offset,
                        ap=[[0, P], list(src.ap[-1])])
                    nc.sync.dma_start(out=bcs[:], in_=bc_in)
                    pairs.append((h, bcs))
                return pairs

            def normalize_phase2(pairs, mul_engine):
                for h, bcs in pairs:
                    c, hf = h // 2, h % 2
                    sl = ct[64 * hf:64 * hf + 64, c, :]
                    if mul_engine == "gpsimd":
                        nc.gpsimd.tensor_mul(sl, sl, bcs[64 * hf:64 * hf + 64, :])
                    else:
                        nc.vector.tensor_mul(sl, sl, bcs[64 * hf:64 * hf + 64, :])

            def final_a(t, crange, start):
                """Output-projection matmuls for query-tile t, chunks crange.

                PSUM: t0 -> pssmall x2, t1 -> pspv x2, t2/t3 -> one psbig
                tile each (2 banks, free once chunk-7 EXPs are done), so all
                four blocks stay live while the last normalize chain runs.
                """
                if t % 4 == 0:
                    y0 = pssmall.tile([P, 512], F32, tag="small", name="y0")
                    y1 = pssmall.tile([P, 512], F32, tag="small", name="y1")
                elif t % 4 == 1:
                    y0 = pspv.tile([P, 512], F32, tag="pv", name="y0",
                                   padded_shape=[P, 512])
                    y1 = pspv.tile([P, 512], F32, tag="pv", name="y1",
                                   padded_shape=[P, 512])
                else:
                    yb = psbig.tile([P, 2, 512], F32, tag="big", name="yb",
                                    padded_shape=[P, 2, 512])
                    y0, y1 = yb[:, 0, :], yb[:, 1, :]
                for eh, yt in ((0, y0), (1, y1)):
                    for i, c in enumerate(crange):
                        nc.tensor.matmul(
                            out=yt[:, 0:512],
                            lhsT=ct[:, c, t * P:(t + 1) * P],
                            rhs=obs[:, c, 512 * eh:512 * (eh + 1)],
                            start=(start and i == 0), stop=False)
                return y0, y1

            def final_b(t, y0, y1):
                for eh, yt in ((0, y0), (1, y1)):
                    nc.tensor.matmul(
                        out=yt[:, 0:512],
                        lhsT=ct[:, 7, t * P:(t + 1) * P],
                        rhs=obs[:, 7, 512 * eh:512 * (eh + 1)],
                        start=False, stop=True)
                ys = ystage.tile([P, E], BF16, tag="ys")
                nc.scalar.copy(ys[:, 0:512], y0[:, 0:512])
                nc.vector.tensor_copy(ys[:, 512:1024], y1[:, 0:512])
                nc.sync.dma_start(out=Y[t * P:(t + 1) * P, :], in_=ys[:])

            # --- software-pipelined main loop --------------------------
            qkt2, pieces = emit_proj_qk_main(0)
            v2 = emit_proj_rest(0, qkt2, pieces)
            cur = (qkt2, v2)
            ytiles = [None] * ntq
            norm_pairs = [None, None, None]
            for c in range(NCH):
                qkt2, v2 = cur
                pt = ppool.tile([P, NG, Lq], BF16, tag="pt")
                sps_live = [None]
                emit_scores(c, qkt2, pt, 0, sps_live)
                if c < NCH - 1:
                    nq = emit_proj_qk_main(c + 1)
                emit_scores(c, qkt2, pt, 1, sps_live)
                if c < NCH - 1:
                    nv2 = emit_proj_rest(c + 1, nq[0], nq[1])
                    cur = (nq[0], nv2)
                else:
                    # batch-1 muls (DVE) hidden under chunk-7 scores, then
                    # the first final block before chunk-7's PV
                    normalize_phase2(norm_pairs[1], "vector")
                    ytiles[0] = final_a(0, list(range(7)), True)
                if c == NORM_BATCHES[0][2] + 1:
                    # batch-0 muls on gpsimd, far off the critical path
                    normalize_phase2(norm_pairs[0], "gpsimd")
                emit_pv(c, pt, v2, 0)
                emit_pv(c, pt, v2, 1)
                for bi, (_, _, bc_) in enumerate(NORM_BATCHES):
                    if c == bc_ and bi < 2:
                        norm_pairs[bi] = normalize_phase1(bi)

            # --- tail: all remaining A-blocks cover the last normalize
            # chain, then the B-blocks (chunk 7) + evacuation ------------
            for t in range(1, ntq):
                ytiles[t] = final_a(t, list(range(7)), True)
            norm_pairs[2] = normalize_phase1(2)
            normalize_phase2(norm_pairs[2], "vector")
            for t in range(ntq):
                final_b(t, *ytiles[t])

    nc.compile()
    return nc


def make_core_inputs(Q, K, V, HeadLinear, OutputLiner, QMask, KMask):
    """Host-side sharding/compaction.

    Returns (in_maps, qidxs, ntq, ntk).  qidxs[b] holds the query
    indices the DEVICE computes (capped at 512; overflow queries are
    computed exactly on the host during gather — see _host_tail).
    """
    bf16 = ml_dtypes.bfloat16
    qm = np.asarray(QMask).astype(bool)
    km = np.asarray(KMask).astype(bool)
    qidxs = [np.nonzero(qm[b])[0] for b in range(B)]
    kidxs = [np.nonzero(km[b])[0] for b in range(B)]
    maxq = max(len(ix) for ix in qidxs)
    qcap = min(maxq, 512)
    qidxs = [ix[:qcap] for ix in qidxs]
    ntq = max(1, math.ceil(max(len(ix) for ix in qidxs) / P))
    ntk = max(1, math.ceil(max(len(ix) for ix in kidxs) / P))
    Lq, Lk = ntq * P, ntk * P

    w2 = np.zeros((P, NCH, P), dtype=np.float32)
    hl = np.asarray(HeadLinear, dtype=np.float32)
    for c in range(NCH):
        w2[0:64, c, 0:64] = hl[2 * c]
        w2[64:128, c, 64:128] = hl[2 * c + 1]
    w2b = w2.astype(bf16)
    ob = np.asarray(OutputLiner, dtype=np.float32).astype(bf16)

    in_maps = []
    for b in range(B):
        qi, ki = qidxs[b], kidxs[b]
        qc = np.zeros((Lq, E), dtype=np.float32)
        qc[:len(qi)] = np.asarray(Q[b], dtype=np.float32)[qi]
        kc = np.zeros((Lk, E), dtype=np.float32)
        kc[:len(ki)] = np.asarray(K[b], dtype=np.float32)[ki]
        vc = np.zeros((Lk, E), dtype=np.float32)
        vc[:len(ki)] = np.asarray(V[b], dtype=np.float32)[ki]
        kmc = np.zeros(Lk, dtype=np.float32)
        kmc[:len(ki)] = 1.0
        in_maps.append({
            "QT": np.ascontiguousarray(qc.T.astype(bf16)),
            "KT": np.ascontiguousarray(kc.T.astype(bf16)),
            "VT": np.ascontiguousarray(vc.T.astype(bf16)),
            "W2": w2b, "OB": ob,
            "KM": np.ascontiguousarray(kmc.reshape(ntk, P).T),
        })
    return in_maps, qidxs, ntq, ntk


_NC_CACHE = {}


def _get_nc(ntq, ntk):
    if (ntq, ntk) not in _NC_CACHE:
        _NC_CACHE[(ntq, ntk)] = build_bass(ntq, ntk)
    return _NC_CACHE[(ntq, ntk)]


def _host_tail(Q, K, V, HeadLinear, OutputLiner, KMask, b, tidx):
    """Exact fp32 attention for a few overflow queries of batch b."""
    hl = np.asarray(HeadLinear, dtype=np.float32)
    ob = np.asarray(OutputLiner, dtype=np.float32)
    ki = np.nonzero(np.asarray(KMask[b]).astype(bool))[0]
    q = np.asarray(Q[b], dtype=np.float32)[tidx]
    kk = np.asarray(K[b], dtype=np.float32)[ki]
    vv = np.asarray(V[b], dtype=np.float32)[ki]
    outs = []
    for h in range(H):
        sl = slice(h * D, (h + 1) * D)
        qh = q[:, sl] @ hl[h]
        kh = kk[:, sl] @ hl[h]
        vh = vv[:, sl] @ hl[h]
        s = (qh @ kh.T) / np.float32(np.sqrt(D))
        s -= s.max(axis=1, keepdims=True)
        p = np.exp(s)
        p /= p.sum(axis=1, keepdims=True)
        outs.append(p @ vh)
    return np.concatenate(outs, axis=1) @ ob


def kernel(Q, K, V, HeadLinear, OutputLiner, QMask, KMask):
    from concourse.bass_utils import run_bass_kernel_spmd

    in_maps, qidxs, ntq, ntk = make_core_inputs(
        Q, K, V, HeadLinear, OutputLiner, QMask, KMask)
    nc = _get_nc(ntq, ntk)
    res = run_bass_kernel_spmd(nc, in_maps, list(range(B)))
    out = np.zeros((B, L, E), dtype=np.float32)
    qm = np.asarray(QMask).astype(bool)
    for b in range(B):
        yc = np.asarray(res.results[b]["Y"]).astype(np.float32)
        out[b][qidxs[b]] = yc[:len(qidxs[b])]
        full = np.nonzero(qm[b])[0]
        tidx = full[len(qidxs[b]):]
        if len(tidx):
            out[b][tidx] = _host_tail(
                Q, K, V, HeadLinear, OutputLiner, KMask, b, tidx)
    return out
